# revision 1
# baseline (speedup 1.0000x reference)
"""Dual attention (DANet-style spatial + channel attention) on 8 Trainium2
NeuronCores.

Sharding: data-parallel over batch B=4, and each batch's output positions
(m in [0, 4096)) split in half across 2 cores -> 8 identical single-core
programs, no collectives. Each core receives its batch's full x (for k/v and
the channel-attention statistics) plus the m-slice of x it owns (for q and
the residual), and produces out[:, m_slice].

Per-core math (x: [512, 4096], m-chunk: 2048 positions):
  spatial:  q=Wq@xq+bq; k=Wk@x+bk; E^T[n,m]=k[:,n].q[:,m]; P=exp(E^T)
            (no max subtraction -- |E| < ~60 so exp fits fp32/bf16 range);
            vT[n,c]=(Wv@x+bv)^T; U[c,m]=sum_n vT[n,c]P[n,m]; Z[m]=sum_n P[n,m]
            s_out = U/Z;  spatial = gamma_s*s_out + xq
  channel:  pT[n,d]=(Wd@x+bd)^T; e=pT^T@pT; c_attn=softmax(rowmax(e)-e);
            c2=gamma_c*(c_attn@p)[:,m]+p[:,m]; channel = Wu@c2+bu
  out = spatial + channel

Performance structure:
  - energy computed TRANSPOSED (n on partitions): exp and the U/Z matmuls
    consume it directly, no [2048,4096] transposes anywhere.
  - fp16 matmuls (1 PE cycle/row; host converts x/weights), bf16 for the
    dominant U matmul (P=exp(E) can reach ~1e24, beyond fp16 range).
    PSUM accumulation is always fp32. fp32 residual path keeps the output
    accurate: ~6e-4 scale-relative absmax vs the fp32 reference.
  - engine split: PE matmuls; ACT exp + bias-adds; DVE softmax-denominator
    accumulation and final combines. Channel output + residual (R) are
    precomputed before the main loop so the per-chunk epilogue is short.
"""
import sys

sys.path.insert(0, '/opt/trn_rl_repo')

import numpy as np

import concourse.bass as bass
import concourse.tile as tile
from concourse import bacc, bass_utils, mybir
from concourse.masks import make_identity

# Problem shapes (fixed by the task spec)
B, C, WIDTH, HEIGHT = 4, 512, 64, 64
N = WIDTH * HEIGHT      # 4096 spatial positions
DK = 64                 # attention inner dim (and channel-attn dim)
NCORES = 8
M = N // 2              # 2048 output positions per core
P = 128
KC = C // P             # 4 input-channel chunks
NT = N // P             # 32 key-position tiles
FREE = 512              # matmul moving free dim (one PSUM bank of fp32)
MCH = M // FREE         # 4 m-chunks per core
CCH = C // P            # 4 output-channel chunks

F32 = mybir.dt.float32
F16 = mybir.dt.float16
BF16 = mybir.dt.bfloat16
AX = mybir.AxisListType
ALU = mybir.AluOpType
ACTF = mybir.ActivationFunctionType

# byte layout of the packed-constants image (per partition)
OFF_WQ, OFF_WD, OFF_WK = 0, 512, 1024
OFF_WU = 1536            # [64, 512] f16, partitions 0-63
OFF_BQ, OFF_BK, OFF_BD = 2560, 2564, 2568
OFF_BU = 2572            # [128, 4] f32
OFF_GC = 2588
OFF_BDR = 2592           # [1, 64] f16 row
OFF_BVR = 2720           # [1, 512] f32 row
OFF_GS = 4768
PKB = 4800


def _bcast_dram(ap, nparts):
    """AP reading a [1]-ish DRAM tensor broadcast across nparts partitions."""
    return bass.AP(tensor=ap.tensor, offset=ap.offset,
                   ap=[[0, nparts], *ap.ap])


def _build_program(tc, io):
    nc = tc.nc
    x_d, xq_d, xqh_d = io['x'], io['xq'], io['xqh']
    out_d = io['out']

    const_cm = tc.tile_pool(name='const', bufs=1)
    const = const_cm.__enter__()

    # ---- persistent SBUF tensors ----
    # All small constants arrive as ONE host-packed byte image (one DMA on
    # the ring instead of 12 serialized descriptors), so the 2MB xqh load --
    # which gates the PE's first matmuls -- starts almost immediately.
    pk_sb = const.tile([P, PKB], mybir.dt.uint8)
    nc.sync.dma_start(pk_sb[:], io['consts'][:])
    wq_sb = pk_sb[:, OFF_WQ:OFF_WQ + 512].bitcast(F16).rearrange(
        "p (kc d) -> p kc d", kc=KC)
    wd_sb = pk_sb[:, OFF_WD:OFF_WD + 512].bitcast(F16).rearrange(
        "p (kc d) -> p kc d", kc=KC)
    wk_sb = pk_sb[:, OFF_WK:OFF_WK + 512].bitcast(F16).rearrange(
        "p (kc d) -> p kc d", kc=KC)
    wu_sb = pk_sb[0:DK, OFF_WU:OFF_WU + 1024].bitcast(F16)
    bq_sb = pk_sb[0:DK, OFF_BQ:OFF_BQ + 4].bitcast(F32)
    bk_sb = pk_sb[0:DK, OFF_BK:OFF_BK + 4].bitcast(F32)
    bd_sb = pk_sb[0:DK, OFF_BD:OFF_BD + 4].bitcast(F32)
    bu_sb = pk_sb[:, OFF_BU:OFF_BU + 16].bitcast(F32)
    gc_sb = pk_sb[0:DK, OFF_GC:OFF_GC + 4].bitcast(F32)
    bdrow_sb = pk_sb[0:1, OFF_BDR:OFF_BDR + 128].bitcast(F16)
    bvrow_sb = pk_sb[0:1, OFF_BVR:OFF_BVR + 2048].bitcast(F32)
    gs_sb = pk_sb[0:1, OFF_GS:OFF_GS + 4].bitcast(F32)

    xqh_sb = const.tile([P, KC, M], F16)   # fp16 matmul operand (first user)
    xqh_r = xqh_d.rearrange("(kc p) m -> p kc m", p=P)
    for mq in range(4):
        qsl = slice(mq * (M // 4), (mq + 1) * (M // 4))
        nc.sync.dma_start(xqh_sb[:, :, qsl], xqh_r[:, :, qsl])
    wv_sb = const.tile([P, KC, C], F16)    # DMA'd below, between x chunks

    ones_row16 = const.tile([1, P], F16)   # lhsT for fp16 rank-1 bias adds
    nc.vector.memset(ones_row16[:], 1.0)
    ones_colb = const.tile([P, 1], BF16)   # lhsT for bf16 partition-sum
    nc.vector.memset(ones_colb[:], 1.0)
    ones_rowb = const.tile([1, P], BF16)   # lhsT for bf16 partition-broadcast
    nc.vector.memset(ones_rowb[:], 1.0)
    ident16 = const.tile([DK, DK], F16)    # for the tiny c_attn transpose
    make_identity(nc, ident16[:])

    k_sb = const.tile([DK, N], F16)        # keys,   [d, n]
    q_sb = const.tile([DK, M], F16)        # queries,[d, m]
    pc_sb = const.tile([DK, M], F16)       # channel proj on the m-slice
    pT_sb = const.tile([P, NT, DK], F16)   # channel proj transposed [n, nt, d]
    vT_sb = const.tile([P, NT, C], BF16)   # values transposed, [n, nt, c]
    c2_sb = const.tile([DK, M], F16)       # gamma_c * c_attn@p + p on m-slice
    bvb_sb = const.tile([P, C], F32)       # bv broadcast to all 128 partitions
    r_sb = const.tile([P, CCH, M], F32)    # channel-out + xq residual

    # ---- phase 1a: q/pc from xqh (PE starts while the big x DMA runs) ----
    with tc.tile_pool(name='ps0', bufs=2, space='PSUM') as ps0:
        for j in range(M // FREE):
            sl = slice(j * FREE, (j + 1) * FREE)
            pq = ps0.tile([DK, FREE], F32, tag='pq')
            for kc in range(KC):
                nc.tensor.matmul(pq[:], lhsT=wq_sb[:, kc],
                                 rhs=xqh_sb[:, kc, sl],
                                 start=(kc == 0), stop=(kc == KC - 1))
            nc.scalar.activation(q_sb[:, sl], pq[:], ACTF.Identity,
                                 bias=bq_sb[:])
            ppc = ps0.tile([DK, FREE], F32, tag='pq')
            for kc in range(KC):
                nc.tensor.matmul(ppc[:], lhsT=wd_sb[:, kc],
                                 rhs=xqh_sb[:, kc, sl],
                                 start=(kc == 0), stop=(kc == KC - 1))
            nc.scalar.activation(pc_sb[:, sl], ppc[:], ACTF.Identity,
                                 bias=bd_sb[:])

    # ---- phase 1b: everything that needs the full x ----
    with tc.tile_pool(name='xp', bufs=1) as xp, \
         tc.tile_pool(name='ps1', bufs=2, space='PSUM') as ps1, \
         tc.tile_pool(name='ps1s', bufs=2, space='PSUM') as ps1s, \
         tc.tile_pool(name='ps1e', bufs=1, space='PSUM') as ps1e:
        x_sb = xp.tile([P, KC, N], F16)
        x_r = x_d.rearrange("(kc p) n -> p kc n", p=P)
        for nq in range(8):
            qsl = slice(nq * (N // 8), (nq + 1) * (N // 8))
            nc.sync.dma_start(x_sb[:, :, qsl], x_r[:, :, qsl])
            if nq == 1:
                # wv lands after the first two x chunks: k-convs (x+wk only)
                # start sooner, vT tiles still find wv ready in time
                nc.sync.dma_start(wv_sb[:],
                                  io['wvT'].rearrange("(kc p) c -> p kc c", p=P))
        xq_sb = xp.tile([P, KC, M], F32)   # fp32 residual source
        nc.sync.dma_start(xq_sb[:], xq_d.rearrange("(kc p) m -> p kc m", p=P))

        # bv broadcast to [128, C] once
        nc.gpsimd.partition_broadcast(bvb_sb[:], bvrow_sb[:], channels=P)

        # k = Wk@x + bk over the full N
        for j in range(N // FREE):
            sl = slice(j * FREE, (j + 1) * FREE)
            pk = ps1.tile([DK, FREE], F32, tag='pk')
            for kc in range(KC):
                nc.tensor.matmul(pk[:], lhsT=wk_sb[:, kc],
                                 rhs=x_sb[:, kc, sl],
                                 start=(kc == 0), stop=(kc == KC - 1))
            nc.scalar.activation(k_sb[:, sl], pk[:], ACTF.Identity,
                                 bias=bk_sb[:])

        # vT and pT tiles: [n, c] = sum_kc x[kc, n]^T W^T[kc, c]  (+ bias)
        for nt in range(NT):
            nsl = slice(nt * P, (nt + 1) * P)
            pv = ps1.tile([P, C], F32, tag='pv')
            for kc in range(KC):
                nc.tensor.matmul(pv[:], lhsT=x_sb[:, kc, nsl],
                                 rhs=wv_sb[:, kc],
                                 start=(kc == 0), stop=(kc == KC - 1))
            nc.vector.tensor_add(vT_sb[:, nt], in0=pv[:], in1=bvb_sb[:])

            pt_ps = ps1s.tile([P, DK], F32, tag='ptp')
            for kc in range(KC):
                nc.tensor.matmul(pt_ps[:], lhsT=x_sb[:, kc, nsl],
                                 rhs=wd_sb[:, kc],
                                 start=(kc == 0), stop=False)
            nc.tensor.matmul(pt_ps[:], lhsT=ones_row16[:], rhs=bdrow_sb[:],
                             start=False, stop=True)
            nc.vector.tensor_copy(pT_sb[:, nt], pt_ps[:])

        # ---- channel attention (tiny): e = pT^T @ pT, softmax, c2, R ----
        with tc.tile_pool(name='sb2', bufs=2) as sb2:
            e_ps = ps1e.tile([DK, DK], F32, tag='e')
            for nt in range(NT):
                nc.tensor.matmul(e_ps[:], lhsT=pT_sb[:, nt], rhs=pT_sb[:, nt],
                                 start=(nt == 0), stop=(nt == NT - 1))
            # c_attn = softmax(rowmax(e) - e) == exp(rowmin(e) - e) / rowsum
            e_sb = sb2.tile([DK, DK], F32, tag='e')
            nc.vector.tensor_copy(e_sb[:], e_ps[:])
            mn_sb = sb2.tile([DK, 1], F32, tag='mn')
            nc.vector.tensor_reduce(mn_sb[:], e_sb[:], axis=AX.X, op=ALU.min)
            h_sb = sb2.tile([DK, DK], F32, tag='h')
            nc.scalar.activation(h_sb[:], e_sb[:], ACTF.Exp,
                                 bias=mn_sb[:], scale=-1.0)
            zc_sb = sb2.tile([DK, 1], F32, tag='zc')
            nc.vector.tensor_reduce(zc_sb[:], h_sb[:], axis=AX.X, op=ALU.add)
            nc.vector.reciprocal(zc_sb[:], zc_sb[:])
            cat16_sb = sb2.tile([DK, DK], F16, tag='cat16')
            nc.vector.tensor_scalar_mul(cat16_sb[:], in0=h_sb[:],
                                        scalar1=zc_sb[:])
            catT_ps = ps1e.tile([DK, DK], F16, tag='catp')
            nc.tensor.transpose(catT_ps[:], cat16_sb[:], ident16[:])
            catT_sb = sb2.tile([DK, DK], F16, tag='cat')
            nc.vector.tensor_copy(catT_sb[:], catT_ps[:])

            # c2 = gamma_c * (c_attn @ p)[:, m_slice] + pc
            for j in range(M // FREE):
                sl = slice(j * FREE, (j + 1) * FREE)
                co_ps = ps1.tile([DK, FREE], F32, tag='pk')
                nc.tensor.matmul(co_ps[:], lhsT=catT_sb[:], rhs=pc_sb[:, sl],
                                 start=True, stop=True)
                nc.vector.scalar_tensor_tensor(
                    out=c2_sb[:, sl], in0=co_ps[:], scalar=gc_sb[:],
                    in1=pc_sb[:, sl], op0=ALU.mult, op1=ALU.add)

            # R = Wu@c2 + bu + xq  (the whole non-spatial part of the output)
            for mc in range(MCH):
                msl = slice(mc * FREE, (mc + 1) * FREE)
                for cc in range(CCH):
                    wu_ps = ps1.tile([P, FREE], F32, tag='pv')
                    nc.tensor.matmul(wu_ps[:],
                                     lhsT=wu_sb[:, cc * P:(cc + 1) * P],
                                     rhs=c2_sb[:, msl], start=True, stop=True)
                    cob_sb = sb2.tile([P, FREE], F32, tag='cob')
                    nc.scalar.activation(cob_sb[:], wu_ps[:], ACTF.Identity,
                                         bias=bu_sb[:, cc:cc + 1])
                    nc.vector.tensor_add(r_sb[:, cc, msl], in0=cob_sb[:],
                                         in1=xq_sb[:, cc, msl])

    # ---- main loop: E^T -> exp -> U/Z accumulation, one m-chunk at a time ----
    out_r = out_d.rearrange("(kc p) m -> p kc m", p=P)
    with tc.tile_pool(name='upool', bufs=4, space='PSUM') as upool, \
         tc.tile_pool(name='epool', bufs=3, space='PSUM') as epool, \
         tc.tile_pool(name='zpool', bufs=1, space='PSUM') as zpool, \
         tc.tile_pool(name='pt', bufs=6) as ptp, \
         tc.tile_pool(name='ssb', bufs=2) as ssb, \
         tc.tile_pool(name='ot', bufs=4) as otp:
        for mc in range(MCH):
            msl = slice(mc * FREE, (mc + 1) * FREE)
            u_ps = [upool.tile([P, FREE], F32, tag='u', name=f'u{mc}_{i}')
                    for i in range(CCH)]
            s_sb = ssb.tile([P, FREE], BF16, tag='s')
            for nt in range(NT):
                nsl = slice(nt * P, (nt + 1) * P)
                e_t = epool.tile([P, FREE], F32, tag='et')
                nc.tensor.matmul(e_t[:], lhsT=k_sb[:, nsl],
                                 rhs=q_sb[:, msl], start=True, stop=True)
                p_t = ptp.tile([P, FREE], BF16, tag='p')
                nc.scalar.activation(p_t[:], e_t[:], ACTF.Exp)
                if nt == 0:
                    nc.vector.tensor_copy(s_sb[:], p_t[:])
                else:
                    nc.vector.tensor_add(s_sb[:], in0=s_sb[:], in1=p_t[:])
                for cc in range(CCH):
                    nc.tensor.matmul(u_ps[cc][:],
                                     lhsT=vT_sb[:, nt, cc * P:(cc + 1) * P],
                                     rhs=p_t[:],
                                     start=(nt == 0), stop=(nt == NT - 1))
            # Z = colsum(S); Zb = gamma_s / Z broadcast to 128 partitions
            z_ps = zpool.tile([1, FREE], F32, tag='z')
            nc.tensor.matmul(z_ps[:], lhsT=ones_colb[:], rhs=s_sb[:],
                             start=True, stop=True)
            zr_sb = ssb.tile([1, FREE], F32, tag='zr')
            nc.vector.reciprocal_approx_fast(out=zr_sb[:], in_=z_ps[:])
            zrb_sb = ssb.tile([1, FREE], BF16, tag='zrb')
            nc.vector.tensor_scalar_mul(zrb_sb[:], in0=zr_sb[:], scalar1=gs_sb[:])
            zb_ps = zpool.tile([P, FREE], F32, tag='z')
            nc.tensor.matmul(zb_ps[:], lhsT=ones_rowb[:], rhs=zrb_sb[:],
                             start=True, stop=True)
            zb_sb = ssb.tile([P, FREE], F32, tag='zb')
            nc.vector.tensor_copy(zb_sb[:], zb_ps[:])
            # combine: out = U*(gamma_s/Z) + R, then store
            for cc in range(CCH):
                o_sb = otp.tile([P, FREE], F32, tag='o')
                nc.vector.tensor_tensor(o_sb[:], u_ps[cc][:], zb_sb[:], ALU.mult)
                nc.vector.tensor_add(o_sb[:], in0=o_sb[:], in1=r_sb[:, cc, msl])
                nc.sync.dma_start(out_r[:, cc, msl], o_sb[:])

    const_cm.__exit__(None, None, None)


_CACHE = {}


def _get_compiled():
    if 'nc' in _CACHE:
        return _CACHE['nc']
    nc = bacc.Bacc("TRN2", num_devices=NCORES)
    io = {
        'x': nc.dram_tensor('x', [C, N], F16, kind='ExternalInput').ap(),
        'xq': nc.dram_tensor('xq', [C, M], F32, kind='ExternalInput').ap(),
        'xqh': nc.dram_tensor('xqh', [C, M], F16, kind='ExternalInput').ap(),
        'wvT': nc.dram_tensor('wvT', [C, C], F16, kind='ExternalInput').ap(),
        'consts': nc.dram_tensor('consts', [P, PKB], mybir.dt.uint8,
                                 kind='ExternalInput').ap(),
        'out': nc.dram_tensor('out', [C, M], F32, kind='ExternalOutput').ap(),
    }
    with tile.TileContext(nc) as tc:
        _build_program(tc, io)
    nc.compile()
    _CACHE['nc'] = nc
    return nc


def make_in_maps(x, Wq, bq, Wk, bk, Wv, bv, gamma_s, Wd, bd, Wu, bu, gamma_c):
    """Build the 8 per-core input dicts from the full problem inputs."""
    f32 = lambda a: np.ascontiguousarray(np.asarray(a, dtype=np.float32))
    f16 = lambda a: np.ascontiguousarray(np.asarray(a, dtype=np.float32)
                                         .astype(np.float16))
    x = f32(x).reshape(B, C, N)

    def w_chunked(wT16):  # [C, DK] f16 -> [128, KC*DK] per-partition bytes
        return np.ascontiguousarray(
            wT16.reshape(KC, P, DK).transpose(1, 0, 2).reshape(P, KC * DK))

    img = np.zeros((P, PKB), np.uint8)
    img[:, OFF_WQ:OFF_WQ + 512] = w_chunked(f16(np.asarray(Wq).T)).view(np.uint8)
    img[:, OFF_WD:OFF_WD + 512] = w_chunked(f16(np.asarray(Wd).T)).view(np.uint8)
    img[:, OFF_WK:OFF_WK + 512] = w_chunked(f16(np.asarray(Wk).T)).view(np.uint8)
    img[0:DK, OFF_WU:OFF_WU + 1024] = f16(np.asarray(Wu).T).view(np.uint8)
    img[0:DK, OFF_BQ:OFF_BQ + 4] = f32(bq)[:, None].view(np.uint8)
    img[0:DK, OFF_BK:OFF_BK + 4] = f32(bk)[:, None].view(np.uint8)
    img[0:DK, OFF_BD:OFF_BD + 4] = f32(bd)[:, None].view(np.uint8)
    img[:, OFF_BU:OFF_BU + 16] = np.ascontiguousarray(
        f32(bu).reshape(CCH, P).T).view(np.uint8)
    img[0:DK, OFF_GC:OFF_GC + 4] = np.broadcast_to(
        f32(gamma_c)[:, None], (DK, 1)).copy().view(np.uint8)
    img[0:1, OFF_BDR:OFF_BDR + 128] = f16(bd)[None, :].view(np.uint8)
    img[0:1, OFF_BVR:OFF_BVR + 2048] = f32(bv)[None, :].view(np.uint8)
    img[0:1, OFF_GS:OFF_GS + 4] = f32(gamma_s)[None, :].view(np.uint8)

    shared = {
        'wvT': f16(np.asarray(Wv).T),
        'consts': img,
    }
    in_maps = []
    for core in range(NCORES):
        b, h = divmod(core, 2)
        xq = x[b][:, h * M:(h + 1) * M]
        in_maps.append({
            'x': f16(x[b]),
            'xq': f32(xq),
            'xqh': f16(xq),
            **shared,
        })
    return in_maps


def assemble_out(results):
    """Stitch the 8 per-core [C, M] outputs back to [B, C, W, H]."""
    full = np.empty((B, C, N), np.float32)
    for core, res in enumerate(results):
        b, h = divmod(core, 2)
        full[b][:, h * M:(h + 1) * M] = res['out']
    return full.reshape(B, C, WIDTH, HEIGHT)


def kernel(**inputs):
    nc = _get_compiled()
    in_maps = make_in_maps(**inputs)
    res = bass_utils.run_bass_kernel_spmd(nc, in_maps, core_ids=list(range(NCORES)))
    return assemble_out(res.results)



# revision 2
# speedup vs baseline: 1.0816x; 1.0816x over previous
"""Dual attention (DANet-style spatial + channel attention) on 8 Trainium2
NeuronCores.

Sharding: data-parallel over batch B=4, each batch's output positions split in
half across 2 cores -> 8 identical single-core programs, no collectives. The
host permutes each core's x so its OWN m-half occupies columns [0, M): softmax
/ sums over n are permutation-invariant, so k/v/p built in permuted order are
fine, and q / residual / output always use columns [0, M).

Per-core math (x: [512, 4096] f16, m-chunk: 2048 positions):
  spatial:  q=Wq@x[:, :M]+bq; k=Wk@x+bk; E^T[n,m]=k[:,n].q[:,m]; P=exp(E^T)
            vT[n,c]=(Wv@x+bv)^T
            U^T[m,c]=sum_n P[n,m] vT[n,c]   (lhsT = P m-slices -> out has m on
            partitions, so 1/Z is a per-PARTITION scalar: no broadcast chain)
            Z[m]=sum_n P[n,m] via 4 tiny matmuls from the DVE-accumulated S
  channel:  pT[n,d]=(Wd@x+bd)^T; e=pT^T@pT; c_attn=softmax(rowmax(e)-e)
            c2=gamma_c*(c_attn@p)[:, :M]+p[:, :M]
            R^T[m,c]=Wu@c2+bu+x[:, :M]  (bu via an appended ones-row: contract
            65; residual from host-transposed xmT f16)
  out^T[m,c] = U^T*(gamma_s/Z)[m] + R^T   -> DRAM [M, C], host transposes.

Performance structure (vs the v0 kernel this replaces):
  - the whole spatial epilogue is per-partition: combine is one
    scalar_tensor_tensor per m-subtile, PSUM banks release via plain DVE
    copies so chunk N+1's U matmuls never wait on the Z chain.
  - channel-attention c2/R^T matmuls are interleaved into chunk 0's PE
    stream -- the PE never idles long enough for HAM to re-throttle (v0 lost
    ~28us to a cold window during the serial channel phase).
  - no xq f32 / xqh inputs: q/pc come straight from the x chunks (own half
    DMA'd first), residual comes from xmT f16. DMA in drops 12.5MB -> 6.6MB.
  - per-partition constant rows (gamma_s, bd-row, bv-row) ship pre-broadcast
    in the packed consts image: no gpsimd broadcasts on the critical path.
"""
import sys

sys.path.insert(0, '/opt/trn_rl_repo')

import numpy as np

import concourse.bass as bass
import concourse.tile as tile
from concourse import bacc, bass_utils, mybir
from concourse.masks import make_identity

# Problem shapes (fixed by the task spec)
B, C, WIDTH, HEIGHT = 4, 512, 64, 64
N = WIDTH * HEIGHT      # 4096 spatial positions
DK = 64                 # attention inner dim (and channel-attn dim)
NCORES = 8
M = N // 2              # 2048 output positions per core
P = 128
KC = C // P             # 4 input-channel chunks
NT = N // P             # 32 key-position tiles
FREE = 512              # matmul moving free dim (one PSUM bank of fp32)
MCH = M // FREE         # 4 m-chunks per core
MS = FREE // P          # 4 m-subtiles (128 rows) per chunk
MT = M // P             # 16 m-subtiles total

F32 = mybir.dt.float32
F16 = mybir.dt.float16
BF16 = mybir.dt.bfloat16
AX = mybir.AxisListType
ALU = mybir.AluOpType
ACTF = mybir.ActivationFunctionType

# byte layout of the packed-constants image (per partition)
OFF_WQ, OFF_WD, OFF_WK = 0, 512, 1024
OFF_WUB = 1536           # [65, 512] f16: rows 0-63 Wu^T, row 64 = bu
OFF_BQ, OFF_BK, OFF_BD, OFF_GC = 2560, 2564, 2568, 2572
OFF_GS = 2576            # [128, 1] f32, replicated on all partitions
OFF_BDB = 2592           # [128, 64] f32, bd row replicated on all partitions
OFF_BVB = 2848           # [128, 512] f32, bv row replicated on all partitions
PKB = 4896


def _build_program(tc, io):
    nc = tc.nc
    x_d, xmT_d, out_d = io['x'], io['xmT'], io['out']

    const_cm = tc.tile_pool(name='const', bufs=1)
    const = const_cm.__enter__()

    # ---- persistent SBUF tensors ----
    pk_sb = const.tile([P, PKB], mybir.dt.uint8)
    nc.sync.dma_start(pk_sb[:], io['consts'][:])
    wq_sb = pk_sb[:, OFF_WQ:OFF_WQ + 512].bitcast(F16).rearrange(
        "p (kc d) -> p kc d", kc=KC)
    wd_sb = pk_sb[:, OFF_WD:OFF_WD + 512].bitcast(F16).rearrange(
        "p (kc d) -> p kc d", kc=KC)
    wk_sb = pk_sb[:, OFF_WK:OFF_WK + 512].bitcast(F16).rearrange(
        "p (kc d) -> p kc d", kc=KC)
    wub_sb = pk_sb[0:DK + 1, OFF_WUB:OFF_WUB + 1024].bitcast(F16)
    bq_sb = pk_sb[0:DK, OFF_BQ:OFF_BQ + 4].bitcast(F32)
    bk_sb = pk_sb[0:DK, OFF_BK:OFF_BK + 4].bitcast(F32)
    bd_sb = pk_sb[0:DK, OFF_BD:OFF_BD + 4].bitcast(F32)
    gc_sb = pk_sb[0:DK, OFF_GC:OFF_GC + 4].bitcast(F32)
    gs_sb = pk_sb[:, OFF_GS:OFF_GS + 4].bitcast(F32)
    bdb_sb = pk_sb[:, OFF_BDB:OFF_BDB + 256].bitcast(F32)
    bvb_sb = pk_sb[:, OFF_BVB:OFF_BVB + 2048].bitcast(F32)

    ones_colb = const.tile([P, 1], BF16)   # rhs for the tiny Z matmuls
    nc.vector.memset(ones_colb[:], 1.0)
    ident16 = const.tile([DK, DK], F16)    # for the tiny c_attn transpose
    make_identity(nc, ident16[:])

    k_sb = const.tile([DK, N], F16)        # keys,   [d, n]
    q_sb = const.tile([DK, M], F16)        # queries,[d, m]
    pc_sb = const.tile([DK, M], F16)       # channel proj on the m-slice
    c2b_sb = const.tile([DK + 1, M], F16)  # c2 rows 0-63, row 64 = ones
    pT_sb = const.tile([P, NT, DK], F16)   # channel proj transposed [n, nt, d]
    vT_sb = const.tile([P, NT, C], BF16)   # values transposed, [n, nt, c]
    catT_sb = const.tile([DK, DK], F16)    # c_attn^T for the c2 matmuls
    xmT_sb = const.tile([P, MT, C], F16)   # residual x^T on the m-slice
    r_sb = const.tile([P, MT, C], F32)     # R^T = channel-out + residual

    nc.vector.memset(c2b_sb[DK:DK + 1, :], 1.0)

    wv_sb = const.tile([P, KC, C], F16)    # DMA'd below, between x chunks

    # ---- phase 1: projections (q/pc from the own half, k/vT/pT from all) ----
    with tc.tile_pool(name='xp', bufs=1) as xp, \
         tc.tile_pool(name='ps0', bufs=2, space='PSUM') as ps0, \
         tc.tile_pool(name='psv', bufs=2, space='PSUM') as psv, \
         tc.tile_pool(name='psp', bufs=2, space='PSUM') as psp, \
         tc.tile_pool(name='psg', bufs=1, space='PSUM') as psg:
        x_sb = xp.tile([P, KC, N], F16)
        x_r = x_d.rearrange("(kc p) n -> p kc n", p=P)
        for nq in range(8):
            qsl = slice(nq * FREE, (nq + 1) * FREE)
            nc.sync.dma_start(x_sb[:, :, qsl], x_r[:, :, qsl])
            if nq == 1:
                nc.sync.dma_start(wv_sb[:],
                                  io['wvT'].rearrange("(kc p) c -> p kc c", p=P))
            if nq == 7:
                nc.sync.dma_start(
                    xmT_sb[:], xmT_d.rearrange("(mt p) c -> p mt c", p=P))

        # q = Wq@x[:, :M] + bq; pc = Wd@x[:, :M] + bd (own half = cols 0..M)
        for j in range(MCH):
            sl = slice(j * FREE, (j + 1) * FREE)
            pq = ps0.tile([DK, FREE], F32, tag='pq')
            for kc in range(KC):
                nc.tensor.matmul(pq[:], lhsT=wq_sb[:, kc],
                                 rhs=x_sb[:, kc, sl],
                                 start=(kc == 0), stop=(kc == KC - 1))
            nc.scalar.activation(q_sb[:, sl], pq[:], ACTF.Identity,
                                 bias=bq_sb[:])
            ppc = ps0.tile([DK, FREE], F32, tag='pq')
            for kc in range(KC):
                nc.tensor.matmul(ppc[:], lhsT=wd_sb[:, kc],
                                 rhs=x_sb[:, kc, sl],
                                 start=(kc == 0), stop=(kc == KC - 1))
            nc.scalar.activation(pc_sb[:, sl], ppc[:], ACTF.Identity,
                                 bias=bd_sb[:])

        # k = Wk@x + bk over the full N
        for j in range(N // FREE):
            sl = slice(j * FREE, (j + 1) * FREE)
            pk = ps0.tile([DK, FREE], F32, tag='pq')
            for kc in range(KC):
                nc.tensor.matmul(pk[:], lhsT=wk_sb[:, kc],
                                 rhs=x_sb[:, kc, sl],
                                 start=(kc == 0), stop=(kc == KC - 1))
            nc.scalar.activation(k_sb[:, sl], pk[:], ACTF.Identity,
                                 bias=bk_sb[:])

        # vT and pT tiles: [n, c] = sum_kc x[kc, n]^T W^T[kc, c]  (+ bias),
        # with the channel gram e += pT[nt]^T pT[nt] accumulated in-loop.
        e_ps = psg.tile([DK, DK], F32, tag='e')
        for nt in range(NT):
            nsl = slice(nt * P, (nt + 1) * P)
            pv = psv.tile([P, C], F32, tag='pv')
            for kc in range(KC):
                nc.tensor.matmul(pv[:], lhsT=x_sb[:, kc, nsl],
                                 rhs=wv_sb[:, kc],
                                 start=(kc == 0), stop=(kc == KC - 1))
            nc.vector.tensor_add(vT_sb[:, nt], in0=pv[:], in1=bvb_sb[:])

            pt_ps = psp.tile([P, DK], F32, tag='ptp')
            for kc in range(KC):
                nc.tensor.matmul(pt_ps[:], lhsT=x_sb[:, kc, nsl],
                                 rhs=wd_sb[:, kc],
                                 start=(kc == 0), stop=(kc == KC - 1))
            nc.vector.tensor_add(pT_sb[:, nt], in0=pt_ps[:], in1=bdb_sb[:])
            nc.tensor.matmul(e_ps[:], lhsT=pT_sb[:, nt], rhs=pT_sb[:, nt],
                             start=(nt == 0), stop=(nt == NT - 1))

        # channel softmax chain (tiny [64, 64] work, overlaps chunk 0 below):
        # c_attn = softmax(rowmax(e) - e) == exp(rowmin(e) - e) / rowsum
        with tc.tile_pool(name='sb2', bufs=1) as sb2:
            e_sb = sb2.tile([DK, DK], F32, tag='e')
            nc.vector.tensor_copy(e_sb[:], e_ps[:])
            mn_sb = sb2.tile([DK, 1], F32, tag='mn')
            nc.vector.tensor_reduce(mn_sb[:], e_sb[:], axis=AX.X, op=ALU.min)
            h_sb = sb2.tile([DK, DK], F32, tag='h')
            nc.scalar.activation(h_sb[:], e_sb[:], ACTF.Exp,
                                 bias=mn_sb[:], scale=-1.0)
            zc_sb = sb2.tile([DK, 1], F32, tag='zc')
            nc.vector.tensor_reduce(zc_sb[:], h_sb[:], axis=AX.X, op=ALU.add)
            nc.vector.reciprocal(zc_sb[:], zc_sb[:])
            cat16_sb = sb2.tile([DK, DK], F16, tag='cat16')
            nc.vector.tensor_scalar_mul(cat16_sb[:], in0=h_sb[:],
                                        scalar1=zc_sb[:])
            catT_ps = psg.tile([DK, DK], F16, tag='catp')
            nc.tensor.transpose(catT_ps[:], cat16_sb[:], ident16[:])
            nc.vector.tensor_copy(catT_sb[:], catT_ps[:])

    # ---- main loop: E^T -> exp -> U^T/Z, one m-chunk at a time.
    # Chunk 0 additionally absorbs the channel c2 + R^T matmuls into its PE
    # stream (their DVE/ACT deps resolve during the early nt iterations).
    out_r = out_d.rearrange("(mt p) c -> p mt c", p=P)
    with tc.tile_pool(name='upool', bufs=4, space='PSUM') as upool, \
         tc.tile_pool(name='epool', bufs=3, space='PSUM') as epool, \
         tc.tile_pool(name='aux', bufs=1, space='PSUM') as aux, \
         tc.tile_pool(name='pt', bufs=6) as ptp, \
         tc.tile_pool(name='ssb', bufs=2) as ssb, \
         tc.tile_pool(name='ot', bufs=10) as otp:

        def channel_tail(step):
            # c2 = gamma_c * (c_attn @ p)[:, :M] + pc  (4 matmuls), then
            # R^T[mt] = (c2 | ones)^T @ (Wu^T | bu) + xmT  (16 matmuls),
            # interleaved into chunk 0 one step per nt iteration.
            if step < MCH:
                j = step
                sl = slice(j * FREE, (j + 1) * FREE)
                co_ps = aux.tile([DK, FREE], F32, tag='aux')
                nc.tensor.matmul(co_ps[:], lhsT=catT_sb[:], rhs=pc_sb[:, sl],
                                 start=True, stop=True)
                nc.vector.scalar_tensor_tensor(
                    out=c2b_sb[0:DK, sl], in0=co_ps[:], scalar=gc_sb[:],
                    in1=pc_sb[:, sl], op0=ALU.mult, op1=ALU.add)
            else:
                mt = step - MCH
                rw_ps = aux.tile([P, C], F32, tag='aux')
                nc.tensor.matmul(rw_ps[:],
                                 lhsT=c2b_sb[:, mt * P:(mt + 1) * P],
                                 rhs=wub_sb[:], start=True, stop=True)
                nc.vector.tensor_add(r_sb[:, mt], in0=rw_ps[:],
                                     in1=xmT_sb[:, mt])

        prev = None  # (u_ps list, s_sb, mc) of the chunk awaiting epilogue

        def epilogue(u_ps, s_sb, mc):
            # Free the U banks first (plain copies, no Z dependency), then
            # combine: out = U^T * (gamma_s/Z) + R^T, all per-partition.
            o1s = []
            for ms in range(MS):
                o1 = otp.tile([P, FREE], F32, tag='o')
                nc.vector.tensor_copy(o1[:], u_ps[ms][:])
                o1s.append(o1)
            z_ps = aux.tile([P, MS], F32, tag='aux')
            for ms in range(MS):
                nc.tensor.matmul(z_ps[:, ms:ms + 1],
                                 lhsT=s_sb[:, ms * P:(ms + 1) * P],
                                 rhs=ones_colb[:],
                                 start=(ms == 0), stop=(ms == MS - 1))
            zr_sb = ssb.tile([P, MS], F32, tag='zr')
            nc.vector.reciprocal(zr_sb[:], z_ps[:])
            zrg_sb = ssb.tile([P, MS], F32, tag='zrg')
            nc.vector.tensor_scalar_mul(zrg_sb[:], in0=zr_sb[:],
                                        scalar1=gs_sb[:])
            for ms in range(MS):
                o2 = otp.tile([P, FREE], F32, tag='o')
                nc.vector.scalar_tensor_tensor(
                    out=o2[:], in0=o1s[ms][:], scalar=zrg_sb[:, ms:ms + 1],
                    in1=r_sb[:, mc * MS + ms], op0=ALU.mult, op1=ALU.add)
                nc.sync.dma_start(out_r[:, mc * MS + ms], o2[:])

        for mc in range(MCH):
            msl = slice(mc * FREE, (mc + 1) * FREE)
            u_ps = [upool.tile([P, FREE], F32, tag='u', name=f'u{mc}_{i}')
                    for i in range(MS)]
            s_sb = ssb.tile([P, FREE], BF16, tag='s')
            for nt in range(NT):
                nsl = slice(nt * P, (nt + 1) * P)
                e_t = epool.tile([P, FREE], F32, tag='et')
                nc.tensor.matmul(e_t[:], lhsT=k_sb[:, nsl],
                                 rhs=q_sb[:, msl], start=True, stop=True)
                p_t = ptp.tile([P, FREE], BF16, tag='p')
                nc.scalar.activation(p_t[:], e_t[:], ACTF.Exp)
                if nt == 0:
                    nc.vector.tensor_copy(s_sb[:], p_t[:])
                    if prev is not None:
                        epilogue(*prev)
                else:
                    nc.vector.tensor_add(s_sb[:], in0=s_sb[:], in1=p_t[:])
                for ms in range(MS):
                    nc.tensor.matmul(u_ps[ms][:],
                                     lhsT=p_t[:, ms * P:(ms + 1) * P],
                                     rhs=vT_sb[:, nt],
                                     start=(nt == 0), stop=(nt == NT - 1))
                if mc == 0 and 6 <= nt < 6 + MCH + MT:
                    channel_tail(nt - 6)
            prev = (u_ps, s_sb, mc)
        epilogue(*prev)

    const_cm.__exit__(None, None, None)


_CACHE = {}


def _get_compiled():
    if 'nc' in _CACHE:
        return _CACHE['nc']
    nc = bacc.Bacc("TRN2", num_devices=NCORES)
    io = {
        'x': nc.dram_tensor('x', [C, N], F16, kind='ExternalInput').ap(),
        'xmT': nc.dram_tensor('xmT', [M, C], F16, kind='ExternalInput').ap(),
        'wvT': nc.dram_tensor('wvT', [C, C], F16, kind='ExternalInput').ap(),
        'consts': nc.dram_tensor('consts', [P, PKB], mybir.dt.uint8,
                                 kind='ExternalInput').ap(),
        'out': nc.dram_tensor('out', [M, C], F32, kind='ExternalOutput').ap(),
    }
    with tile.TileContext(nc) as tc:
        _build_program(tc, io)
    nc.compile()
    _CACHE['nc'] = nc
    return nc


def make_in_maps(x, Wq, bq, Wk, bk, Wv, bv, gamma_s, Wd, bd, Wu, bu, gamma_c):
    """Build the 8 per-core input dicts from the full problem inputs."""
    f32 = lambda a: np.ascontiguousarray(np.asarray(a, dtype=np.float32))
    f16 = lambda a: np.ascontiguousarray(np.asarray(a, dtype=np.float32)
                                         .astype(np.float16))
    x = f32(x).reshape(B, C, N)

    def w_chunked(wT16):  # [C, DK] f16 -> [128, KC*DK] per-partition bytes
        return np.ascontiguousarray(
            wT16.reshape(KC, P, DK).transpose(1, 0, 2).reshape(P, KC * DK))

    img = np.zeros((P, PKB), np.uint8)
    img[:, OFF_WQ:OFF_WQ + 512] = w_chunked(f16(np.asarray(Wq).T)).view(np.uint8)
    img[:, OFF_WD:OFF_WD + 512] = w_chunked(f16(np.asarray(Wd).T)).view(np.uint8)
    img[:, OFF_WK:OFF_WK + 512] = w_chunked(f16(np.asarray(Wk).T)).view(np.uint8)
    wub = np.concatenate([f16(np.asarray(Wu).T), f16(bu)[None, :]], axis=0)
    img[0:DK + 1, OFF_WUB:OFF_WUB + 1024] = np.ascontiguousarray(wub).view(np.uint8)
    img[0:DK, OFF_BQ:OFF_BQ + 4] = f32(bq)[:, None].view(np.uint8)
    img[0:DK, OFF_BK:OFF_BK + 4] = f32(bk)[:, None].view(np.uint8)
    img[0:DK, OFF_BD:OFF_BD + 4] = f32(bd)[:, None].view(np.uint8)
    img[0:DK, OFF_GC:OFF_GC + 4] = np.broadcast_to(
        f32(gamma_c)[:, None], (DK, 1)).copy().view(np.uint8)
    img[:, OFF_GS:OFF_GS + 4] = np.broadcast_to(
        f32(gamma_s)[:, None], (P, 1)).copy().view(np.uint8)
    img[:, OFF_BDB:OFF_BDB + 256] = np.broadcast_to(
        f32(bd)[None, :], (P, DK)).copy().view(np.uint8)
    img[:, OFF_BVB:OFF_BVB + 2048] = np.broadcast_to(
        f32(bv)[None, :], (P, C)).copy().view(np.uint8)

    shared = {
        'wvT': f16(np.asarray(Wv).T),
        'consts': img,
    }
    in_maps = []
    for core in range(NCORES):
        b, h = divmod(core, 2)
        own = slice(h * M, (h + 1) * M)
        other = slice((1 - h) * M, (2 - h) * M)
        xp = np.concatenate([x[b][:, own], x[b][:, other]], axis=1)
        in_maps.append({
            'x': f16(xp),
            'xmT': f16(x[b][:, own].T),
            **shared,
        })
    return in_maps


def assemble_out(results):
    """Stitch the 8 per-core [M, C] outputs back to [B, C, W, H]."""
    full = np.empty((B, C, N), np.float32)
    for core, res in enumerate(results):
        b, h = divmod(core, 2)
        full[b][:, h * M:(h + 1) * M] = res['out'].T
    return full.reshape(B, C, WIDTH, HEIGHT)


def kernel(**inputs):
    nc = _get_compiled()
    in_maps = make_in_maps(**inputs)
    res = bass_utils.run_bass_kernel_spmd(nc, in_maps, core_ids=list(range(NCORES)))
    return assemble_out(res.results)
